# revision 8
# baseline (speedup 1.0000x reference)
"""Trainium2 Bass kernel for nn_KANActivation (pykan-style KAN activation).

y[b,c,h,w] = scale_base[c]*silu(x) + scale_sp[c]*sum_k coef[c,k]*B_k(x) + bias[c]

where B_k are cubic B-spline basis functions on the uniform extended grid
knots x_m = -2.2 + 0.4*m (m = 0..11).

Math used by the kernel (exact identities, verified numerically):
  spline_c(x) = sum_m g[c,m] * relu(x - x_m)^3 / h^3        (truncated powers)
  sum_m g[c,m] * (x - x_m)^3 == 0  identically               (spline has compact
                                                              support on the right)
  => spline_c(x) = 0.5 * sum_m ghat[c,m] * |x - x_m|^3       (globally exact)
  |x - x_m|^3 = (x^2 - 2 x_m x + x_m^2) * |x - x_m|
  => spline_c(x) = x^2*S2 + x*S1 + S0  with  S_d = sum_m c_d[c,m] * |x - x_m|

so per element we need 12 abs-features and three per-channel weighted sums of
them, plus a silu.  Sharding: batch B=32 -> 4 per core across 8 cores; the
tiny per-channel params are replicated.
"""

import os
from math import comb

import numpy as np

import concourse.bass as bass
import concourse.bacc as bacc
import concourse.tile as tile
from concourse import mybir
from concourse.bass_utils import run_bass_kernel_spmd

F32 = mybir.dt.float32
ALU = mybir.AluOpType
ACT = mybir.ActivationFunctionType

NCORES = 8
B, C, H, W = 32, 64, 128, 128
HW = H * W                      # 16384
B_LOC = B // NCORES             # 4 batches per core
FT = 1024                       # free-dim per half tile; a tile covers 2*FT cols
N_M = 12                        # number of knots / abs features
XM = [-2.2 + 0.4 * m for m in range(N_M)]
# param columns: c2[12] c1[12] c0[12] sb bias negxm[12]
NPARAM = 4 * N_M + 2


def _build_params(coef, scale_base, scale_sp, bias):
    """fp64 host precompute -> [128, NPARAM] fp32 (rows duplicated x2)."""
    coef64 = np.asarray(coef, np.float64)
    g = np.zeros((C, N_M))
    for m in range(N_M):
        for i in range(5):
            k = m - i
            if 0 <= k <= 7:
                g[:, m] += (-1) ** i * comb(4, i) * coef64[:, k] / 6.0
    xm = np.asarray(XM)
    # 0.5 * (1/h^3) * scale_sp * g ; h = 0.4
    G = 0.5 * 15.625 * np.asarray(scale_sp, np.float64)[:, None] * g
    c2 = G
    c1 = -2.0 * xm[None, :] * G
    c0 = (xm ** 2)[None, :] * G
    p = np.concatenate(
        [c2, c1, c0,
         np.asarray(scale_base, np.float64)[:, None],
         np.asarray(bias, np.float64)[:, None],
         np.tile(-xm[None, :], (C, 1))],
        axis=1,
    ).astype(np.float32)                       # [64, NPARAM]
    # partition p holds channel p//2 (tiles are loaded as (c, k) interleaved)
    return np.repeat(p, 2, axis=0)             # [128, NPARAM]


def _build_bass():
    nc = bacc.Bacc(trn_type="TRN2")
    x_d = nc.dram_tensor("x", [B_LOC, C, HW], F32, kind="ExternalInput")
    pp_d = nc.dram_tensor("pp", [128, NPARAM], F32, kind="ExternalInput")
    out_d = nc.dram_tensor("out", [B_LOC, C, HW], F32, kind="ExternalOutput")

    with tile.TileContext(nc) as tc:
        with tc.tile_pool(name="const", bufs=1) as constp, \
             tc.tile_pool(name="io", bufs=3) as iop, \
             tc.tile_pool(name="work", bufs=2) as wp:
            pp = constp.tile([128, NPARAM], F32)
            nc.sync.dma_start(out=pp, in_=pp_d[:, :])

            def col(j):
                return pp[:, j:j + 1]

            sb_ap, bias_ap = col(3 * N_M), col(3 * N_M + 1)

            for b in range(B_LOC):
                for j in range(HW // (2 * FT)):
                    f0 = 2 * FT * j
                    xt = iop.tile([128, FT], F32, tag="xt")
                    nc.sync.dma_start(
                        out=xt,
                        in_=x_d[b, :, f0:f0 + 2 * FT].rearrange(
                            "c (k f) -> c k f", k=2),
                    )

                    xc = wp.tile([128, FT], F32, tag="xc")
                    nc.vector.tensor_scalar(xc, xt, 2.2, -2.2, ALU.min, ALU.max)

                    sl = wp.tile([128, FT], F32, tag="sl")
                    nc.scalar.activation(sl, xt, ACT.Silu)
                    o0 = wp.tile([128, FT], F32, tag="o0")
                    nc.vector.tensor_scalar(o0, sl, sb_ap, bias_ap,
                                            ALU.mult, ALU.add)

                    S2 = S1 = S0 = None
                    for m in range(N_M):
                        A = wp.tile([128, FT], F32, tag="A")
                        nc.scalar.activation(A, xc, ACT.Abs, bias=col(3 * N_M + 2 + m))
                        if m == 0:
                            S2 = wp.tile([128, FT], F32, tag="S2")
                            S1 = wp.tile([128, FT], F32, tag="S1")
                            S0 = wp.tile([128, FT], F32, tag="S0")
                            nc.vector.tensor_scalar_mul(S2, A, col(m))
                            nc.vector.tensor_scalar_mul(S1, A, col(N_M + m))
                            nc.vector.tensor_scalar_mul(S0, A, col(2 * N_M + m))
                        else:
                            S2n = wp.tile([128, FT], F32, tag="S2")
                            S1n = wp.tile([128, FT], F32, tag="S1")
                            S0n = wp.tile([128, FT], F32, tag="S0")
                            nc.vector.scalar_tensor_tensor(
                                S2n, A, col(m), S2, ALU.mult, ALU.add)
                            nc.vector.scalar_tensor_tensor(
                                S1n, A, col(N_M + m), S1, ALU.mult, ALU.add)
                            nc.vector.scalar_tensor_tensor(
                                S0n, A, col(2 * N_M + m), S0, ALU.mult, ALU.add)
                            S2, S1, S0 = S2n, S1n, S0n

                    h1 = wp.tile([128, FT], F32, tag="h1")
                    nc.vector.scalar_tensor_tensor(h1, xc, 1.0, S2,
                                                   ALU.mult, ALU.mult)
                    h2 = wp.tile([128, FT], F32, tag="h2")
                    nc.vector.tensor_add(h2, h1, S1)
                    h3 = wp.tile([128, FT], F32, tag="h1")
                    nc.vector.scalar_tensor_tensor(h3, xc, 1.0, h2,
                                                   ALU.mult, ALU.mult)
                    h4 = wp.tile([128, FT], F32, tag="h2")
                    nc.vector.tensor_add(h4, h3, S0)
                    of = wp.tile([128, FT], F32, tag="of")
                    nc.vector.tensor_add(of, o0, h4)

                    nc.sync.dma_start(
                        out=out_d[b, :, f0:f0 + 2 * FT].rearrange(
                            "c (k f) -> c k f", k=2),
                        in_=of,
                    )
    nc.finalize()
    return nc


_CACHE = {}


def _run(x, coef, scale_base, scale_sp, bias, trace=False):
    if "nc" not in _CACHE:
        _CACHE["nc"] = _build_bass()
    nc = _CACHE["nc"]
    params = _build_params(coef, scale_base, scale_sp, bias)
    xs = np.ascontiguousarray(x, np.float32).reshape(NCORES, B_LOC, C, HW)
    in_maps = [{"x": xs[i], "pp": params} for i in range(NCORES)]
    res = run_bass_kernel_spmd(nc, in_maps, list(range(NCORES)), trace=trace)
    out = np.stack([r["out"] for r in res.results], axis=0)
    return out.reshape(B, C, H, W), res


def kernel(x, coef, scale_base, scale_sp, bias):
    out, _ = _run(x, coef, scale_base, scale_sp, bias,
                  trace=bool(int(os.environ.get("KAN_TRACE", "0"))))
    return out
